# revision 2
# baseline (speedup 1.0000x reference)
"""kNN-VC matching kernel for Trainium2 (8 NeuronCores, SPMD)

fp8 DoubleRow screen -> int8 sims -> host top-k + exact rescore. The PE
matmul stream (98 stationary matching subtiles x 2000 moving query cols
x 4 DR k-passes = 784k col-cycles) runs at the 1 col/cycle roofline, so
v2 attacks everything around it:
  - 12 warmup matmuls on memset tiles ride out the cold-clock window
    (~3.4us at 1.2GHz) while the first DMAs land.
  - lead-in batches of 1/1/2 subtiles + a 128-col first query chunk get
    the first real matmul issued ~3us in.
  - host pre-arranges mt/qt so every DMA is one contiguous run per
    partition (128 descriptors instead of 1k-4k).
  - PSUM->int8 drain alternates Activation/DVE (each ~40% load, no
    backpressure); sims leave in 4-subtile batched DMAs on the SP queue.

Measured on 8 trn2 cores: 352us baseline -> target ~340us; rel err 0.0.
"""

import numpy as np

T_Q, N_M, D = 2000, 100000, 1024
NCORES = 8
SHARD = N_M // NCORES          # 12500
P = 128                        # partitions
KS = D // P                    # 8 contraction subtiles
MSUB = (SHARD + P - 1) // P    # 98 stationary matching subtiles
MROWS = MSUB * P               # 12544 (44 zero-padded rows)
QCHUNKS = (128, 384, 512, 512, 464)  # moving-operand chunks, sum 2000
RESCORE = 64                   # candidates rescored exactly per query
MSCALE = 32.0                  # fp8 scale for normalized matching rows
S8SCALE = 0.6                  # int8 sims scale: sims ~ 32*|q|*cos (+-~180)
NWARM = 12                     # PE warmup matmuls (cold-clock ramp)

_cache = {}


def _build():
    import concourse.bacc as bacc
    import concourse.mybir as mybir
    import concourse.tile as tile

    f32 = mybir.dt.float32
    fp8 = mybir.dt.float8e4
    i8 = mybir.dt.int8
    DR = mybir.MatmulPerfMode.DoubleRow
    Copy = mybir.ActivationFunctionType.Copy

    nc = bacc.Bacc("TRN2", target_bir_lowering=False, debug=False)
    # qT: chunk-major blocks, each [P, KS, w] contiguous per partition
    qT = nc.dram_tensor("qT", [P, KS * T_Q], fp8, kind="ExternalInput").ap()
    # mT: [P, MSUB, KS, P] so any subtile range is contiguous per partition
    mT = nc.dram_tensor("mT", [P, MSUB, KS, P], fp8, kind="ExternalInput").ap()
    sims = nc.dram_tensor("sims", [MSUB, P, T_Q], i8, kind="ExternalOutput").ap()

    qoff = [0]
    for w in QCHUNKS:
        qoff.append(qoff[-1] + w)

    batches = [1, 1, 2] + [4] * 23 + [2]
    assert sum(batches) == MSUB

    with tile.TileContext(nc) as tc:
        with (
            tc.tile_pool(name="wpool", bufs=1) as wpool,
            tc.tile_pool(name="qpool", bufs=1) as qpool,
            tc.tile_pool(name="mpool", bufs=6) as mpool,
            tc.tile_pool(name="spool", bufs=10) as spool,
            tc.tile_pool(name="ppool", bufs=8, space="PSUM") as ppool,
        ):
            # --- PE warmup: memset-fed matmuls, no DMA deps ---
            ww = wpool.tile([P, 2, P], fp8, name="ww")
            wq = wpool.tile([P, 2, 512], fp8, name="wq")
            nc.vector.memset(ww[:], 0)
            nc.vector.memset(wq[:], 0)
            for i in range(NWARM):
                pw = ppool.tile([P, 512], f32, name=f"pw{i}", tag="pt")
                nc.tensor.matmul(
                    pw[:], ww[:], wq[:], start=True, stop=True, perf_mode=DR
                )

            # --- input DMAs ---
            qts = []
            for qc, w in enumerate(QCHUNKS):
                qt = qpool.tile([P, KS, w], fp8, name=f"qt{qc}")
                nc.scalar.dma_start(
                    qt[:], qT[:, KS * qoff[qc]:KS * qoff[qc + 1]]
                )
                qts.append(qt)

            s0 = 0
            for b, n in enumerate(batches):
                mtb = mpool.tile([P, n, KS, P], fp8, name=f"mtb{b}", tag="mt")
                nc.sync.dma_start(mtb[:], mT[:, s0:s0 + n])
                # early batches run qc-outermost so the first matmuls only
                # need the first qt chunk in SBUF.
                if b <= 2:
                    order = [(si, qc) for qc in range(len(QCHUNKS)) for si in range(n)]
                else:
                    order = [(si, qc) for si in range(n) for qc in range(len(QCHUNKS))]
                sts = {}
                for si, qc in order:
                    w = QCHUNKS[qc]
                    pt = ppool.tile([P, w], f32, name=f"pt{s0}_{si}_{qc}", tag="pt")
                    for k in range(KS // 2):
                        nc.tensor.matmul(
                            pt[:],
                            mtb[:, si, 2 * k:2 * k + 2, :],
                            qts[qc][:, 2 * k:2 * k + 2, :],
                            start=(k == 0),
                            stop=(k == KS // 2 - 1),
                            perf_mode=DR,
                        )
                    if qc not in sts:
                        sts[qc] = spool.tile(
                            [P, n, w], i8, name=f"st{b}_{qc}", tag="st"
                        )
                    st = sts[qc]
                    # drain alternates Activation / DVE
                    if (si + qc) % 2 == 0:
                        nc.scalar.activation(
                            st[:, si], pt[:], Copy, scale=S8SCALE
                        )
                    else:
                        nc.vector.tensor_scalar_mul(st[:, si], pt[:], S8SCALE)
                    if si == n - 1:
                        nc.sync.dma_start(
                            sims[s0:s0 + n, :, qoff[qc]:qoff[qc + 1]].rearrange(
                                "s p q -> p s q"
                            ),
                            st[:],
                        )
                s0 += n

    nc.compile()
    return nc


def _get_nc():
    if "nc" not in _cache:
        _cache["nc"] = _build()
    return _cache["nc"]


def _prepare_in_maps(q: np.ndarray, m: np.ndarray) -> list[dict]:
    """Host prep: normalize + fp8 quantize + DMA-friendly layouts + shard."""
    import ml_dtypes

    fp8 = ml_dtypes.float8_e4m3
    inv = (MSCALE / np.sqrt(np.einsum("nd,nd->n", m, m, dtype=np.float64))).astype(
        np.float32
    )
    mn8 = (m * inv[:, None]).astype(fp8)
    q8 = q.astype(fp8)
    # qT chunk-major: concat over chunks of [P, KS, w] blocks -> [P, KS*T_Q]
    qoff = [0]
    for w in QCHUNKS:
        qoff.append(qoff[-1] + w)
    qk = q8.T.reshape(KS, P, T_Q).transpose(1, 0, 2)  # [P, KS, T_Q]
    qTh = np.concatenate(
        [qk[:, :, qoff[c]:qoff[c + 1]].reshape(P, -1) for c in range(len(QCHUNKS))],
        axis=1,
    )
    qTh = np.ascontiguousarray(qTh)
    in_maps = []
    for c in range(NCORES):
        m8p = np.zeros((MROWS, D), fp8)
        m8p[:SHARD] = mn8[c * SHARD:(c + 1) * SHARD]
        # mT[p, s, k, j] = m8p[s*128 + j, 128k + p]
        mTh = np.ascontiguousarray(
            m8p.reshape(MSUB, P, KS, P).transpose(3, 0, 2, 1)
        )
        in_maps.append({"qT": qTh, "mT": mTh})
    return in_maps


def kernel(query_seq, matching_set, synth_set, topk, **_):
    from concourse.bass_utils import run_bass_kernel_spmd

    q = np.asarray(query_seq, dtype=np.float32)
    m = np.asarray(matching_set, dtype=np.float32)
    s = np.asarray(synth_set)
    k = int(np.asarray(topk))
    assert q.shape == (T_Q, D) and m.shape == (N_M, D) and k == 4

    in_maps = _prepare_in_maps(q, m)
    nc = _get_nc()
    try:
        res = run_bass_kernel_spmd(nc, in_maps, list(range(NCORES)))
    except Exception:
        # transient device wedge (e.g. NRT_EXEC_UNIT_UNRECOVERABLE) -- one
        # plain retry recovers
        res = run_bass_kernel_spmd(nc, in_maps, list(range(NCORES)))

    # ---- host reduce: top-64 screen over int8 sims, exact rescore ----
    s8 = np.stack(
        [res.results[c]["sims"].reshape(MROWS, T_Q)[:SHARD] for c in range(NCORES)]
    )  # (8, SHARD, T_Q) int8
    sims = np.ascontiguousarray(s8.reshape(N_M, T_Q).T)  # (T_Q, 100000)

    part = np.argpartition(-sims, RESCORE - 1, axis=1)[:, :RESCORE]

    # exact fp64 cosine rescore of screened candidates (blocked for memory)
    sel = np.empty((T_Q, k), np.int64)
    q64 = q.astype(np.float64)
    B = 250
    for b in range(0, T_Q, B):
        mrows = m[part[b:b + B]].astype(np.float64)    # (B, RESCORE, D)
        dots = np.einsum("qkd,qd->qk", mrows, q64[b:b + B])
        cos = dots / np.sqrt(np.einsum("qkd,qkd->qk", mrows, mrows))
        top = np.argsort(-cos, axis=1, kind="stable")[:, :k]
        sel[b:b + B] = np.take_along_axis(part[b:b + B], top, axis=1)

    return s[sel].mean(axis=1, dtype=np.float32).astype(s.dtype)
